# revision 23
# baseline (speedup 1.0000x reference)
"""Trainium2 kernel for nn_ContinuousLocationMap.

Contract: kernel(locs) with locs [8, 1024, 2] f32 -> [8, 2045, 2045, 4] f32.

Per batch item b the output grid is
    out[i, j] = hit(i,j) ? [1, 1, locs[b,w,0], locs[b,w,1]]
                         : [0.634, 0.634, xs[j], xs[i]]
where hit cells come from a 2x2 scatter window around each location index
and w is the last (max-ordinal) location that touched the cell.

Strategy (pure data parallel, one batch item per NeuronCore):
  host:   resolve the scatter winners (<= 4096 cells per item), cover the
          hit-cell set with ~2048 adjacent-cell PAIRS (32 B writes); every
          written half carries the cell's FINAL value, so overlapping or
          padded writes are order-independent.  Pairs are packed, in cell
          order, into NOPS=17 indirect-DMA ops of 128 slots such that op o
          only holds pairs whose rows lie in 128-row tiles <= min(o, 15).
  device: generate the base-map template on-device (~7 us on gpsimd+DVE,
          no 4.2 MB HBM template loads contending with stores), stream 16
          contiguous ~4.2 MB template stores at the write roofline
          (stride-4 y-channel rewrite per tile hidden on DVE), and patch
          hit cells with banded indirect DMAs: scatter op o waits only on
          the store of tile o, so patching hides under the remaining dense
          stores instead of tailing ~35 us after the last one.
"""

import numpy as np

# ---- hyperparameters (must mirror reference.py) ----
MIN_LOC = 0.0
MAX_LOC = 512.0
BINS = 2048
STRIDE = 1
WINDOW = 5

LOC_DELTA = (MAX_LOC - MIN_LOC) / BINS            # 0.25
WSIDE = WINDOW // 2                                # 2
BINS_WINDOW = BINS - 2 * WSIDE                     # 2044
MIN_W = MIN_LOC + LOC_DELTA * WSIDE                # 0.5
MAX_W = MIN_LOC + LOC_DELTA * BINS_WINDOW          # 511.0
G = int((BINS_WINDOW + 1) // STRIDE)               # 2045
DELTA_MAP = (MAX_W - MIN_W) / G
CORR_BASE = 0.634

BATCH = 8
N_LOCS = 1024

P = 128                       # SBUF partitions
ROWF = G * 4                  # 8180 floats per output row
NT = (G + P - 1) // P         # 16 row-tiles; last has 125 rows
NOPS = NT + 1                 # scatter ops: one per tile + one spill op
NPAIR = NOPS * P              # 2176 pair slots (>= 2048 = worst-case pairs)

TRACE = False                 # test.py sets this for profiling runs
LAST_RESULT = None            # BassKernelResults from the last run

# Bumped on every program-affecting edit.  Salts a dummy-output shape so
# the PJRT/NEFF compile-cache fingerprint can never serve a stale NEFF
# built from an older kernel version with identical tensor shapes.
KERNEL_VERSION = 5

_XS = (np.float32(MIN_W)
       + np.float32(DELTA_MAP) * np.arange(G, dtype=np.float32)).astype(np.float32)

_OFFS = np.array([[-1, -1], [-1, 0], [0, -1], [0, 0]], dtype=np.int32)


def _host_shared_inputs():
    """Template base tile (tile 0 content) and per-tile y columns."""
    btile = np.empty((P, ROWF), dtype=np.float32)
    btile[:, 0::4] = CORR_BASE
    btile[:, 1::4] = CORR_BASE
    btile[:, 2::4] = _XS[None, :]
    btile[:, 3::4] = _XS[:P, None]

    ycols = np.empty((P, NT), dtype=np.float32)
    for t in range(NT):
        rows = np.minimum(t * P + np.arange(P), G - 1)
        ycols[:, t] = _XS[rows]
    return btile, ycols


def _host_scatter(locs_b):
    """Pair-cover the last-write-wins scatter for one batch item.

    Returns (hidx [P, NOPS] int32, hval [P, NOPS*8] f32): op o scatters,
    per partition s, 8 floats hval[s, 8o:8o+8] to cells (c, c+1) where
    c = hidx[s, o].  Both halves always carry the cells' FINAL values
    (hit value or base-map value), so any overlap/padding is idempotent.
    Op o only contains pairs whose max touched row is in tile <= min(o,
    NT-1); the device schedules op o after the store of that tile.
    """
    locs_b = np.asarray(locs_b, dtype=np.float32)
    idx = (locs_b / np.float32(LOC_DELTA) / np.float32(STRIDE)).astype(np.int32)
    pos = idx[:, None, :] + _OFFS[None, :, :]                   # [L, 4, 2]
    valid = np.all((pos >= 0) & (pos <= G - 1), axis=-1)        # [L, 4]
    flat = np.where(valid, pos[..., 0] * G + pos[..., 1], 0)
    ordn = np.where(valid, np.arange(locs_b.shape[0], dtype=np.int64)[:, None], -1)

    winner = np.full(G * G, -1, dtype=np.int64)
    np.maximum.at(winner, flat.ravel(), ordn.ravel())
    cells = np.unique(flat[valid])
    cells = cells[winner[cells] >= 0]                           # ascending

    # greedy pair cover: lone trailing cell G*G-1 shifts left to stay
    # in-bounds (its left neighbour's final value is known anyway)
    c0s = []
    i, k = 0, len(cells)
    while i < k:
        c = int(cells[i])
        if i + 1 < k and cells[i + 1] == c + 1:
            c0s.append(c)
            i += 2
        else:
            c0s.append(c if c + 1 < G * G else c - 1)
            i += 1
    c0s = np.asarray(c0s + [0], dtype=np.int64)                 # +pad template
    npair = len(c0s) - 1
    assert npair <= NPAIR, npair

    def final_vals(cs):
        w = winner[cs]
        hit = w >= 0
        v = np.empty((len(cs), 4), np.float32)
        v[:, 0] = np.where(hit, np.float32(1.0), np.float32(CORR_BASE))
        v[:, 1] = v[:, 0]
        v[:, 2] = np.where(hit, locs_b[np.clip(w, 0, None), 0], _XS[cs % G])
        v[:, 3] = np.where(hit, locs_b[np.clip(w, 0, None), 1], _XS[cs // G])
        return v

    v8 = np.concatenate([final_vals(c0s), final_vals(c0s + 1)], axis=1)  # [n+1, 8]
    band = np.minimum(((c0s + 1) // G) // P, NT - 1)            # tile of max row

    # FIFO fill: op o takes the next <=128 pairs with band <= min(o, NT-1);
    # unused slots replay the pad pair (index npair: cell 0, final values).
    # If the banded schedule cannot place every pair (pathological row
    # clustering), fall back to unbanded fill (every op may hold any pair;
    # the device then runs every scatter after the last store).
    banded_ok = True
    sel = np.full((P, NOPS), npair, dtype=np.int64)
    ptr = 0
    for o in range(NOPS):
        bmax = min(o, NT - 1)
        take = 0
        while ptr < npair and take < P and band[ptr] <= bmax:
            sel[take, o] = ptr
            ptr += 1
            take += 1
    if ptr != npair:
        banded_ok = False
        sel = np.full((P, NOPS), npair, dtype=np.int64)
        order = np.arange(npair)
        sel[order % P, order // P] = order   # NOPS*P = 2176 >= max pairs

    hidx = c0s[sel].astype(np.int32)                            # [P, NOPS]
    # v8[sel] is [P, NOPS, 8]; hval[s, 8o:8o+8] = v8[sel[s, o]]
    hval = np.ascontiguousarray(v8[sel]).reshape(P, NOPS * 8).astype(np.float32)
    _host_scatter.last_banded_ok = banded_ok
    return hidx, hval


_NC_CACHE = None


def _build_nc(repeat=1, *, scatter=True, serialize_reps=False, store_queues=1,
              gen_template=True, banded=True):
    """Build the per-core Bass program (same program on all 8 cores).

    repeat>1 unrolls the whole store+scatter pipeline N times inside one
    NEFF (idempotent rewrites) — used by bench.py to isolate steady-state
    device time from the ~200ms per-call PJRT/axon dispatch overhead.
    Keyword-only flags are experiment knobs; defaults are the production
    config.
    """
    from concourse import bass, bacc, mybir
    import concourse.tile as tile
    from concourse.tile import add_dep_helper

    nc = bacc.Bacc(None, target_bir_lowering=False)
    f32 = mybir.dt.float32
    btile = None if gen_template else \
        nc.dram_tensor("btile", [P, ROWF], f32, kind="ExternalInput")
    ycols = nc.dram_tensor("ycols", [P, NT], f32, kind="ExternalInput")
    hidx = nc.dram_tensor("hidx", [P, NOPS], mybir.dt.int32, kind="ExternalInput")
    hval = nc.dram_tensor("hval", [P, NOPS * 8], f32, kind="ExternalInput")
    out = nc.dram_tensor("out", [G * G, 4], f32, kind="ExternalOutput")
    out_rows = out[:].rearrange("(g w) c -> g (w c)", w=G)      # [G, ROWF]
    # cache-buster: never-written dummy output whose shape encodes the
    # kernel version and build flags (see KERNEL_VERSION comment).
    salt = (KERNEL_VERSION * 1009 + repeat * 13 + int(scatter) * 7
            + int(serialize_reps) * 3 + store_queues
            + int(gen_template) * 29 + int(banded) * 61) % 8191 + 1
    nc.dram_tensor("vsalt", [1, salt], f32, kind="ExternalOutput")

    with tile.TileContext(nc) as tc:
        with tc.tile_pool(name="big", bufs=1) as big, \
             tc.tile_pool(name="small", bufs=1) as small:
            yc = small.tile([P, NT], f32, tag="yc")
            hi = small.tile([P, NOPS], mybir.dt.int32, tag="hi")
            hv = small.tile([P, NOPS * 8], f32, tag="hv")

            buf_a = big.tile([P, ROWF], f32, tag="bufA")
            buf_b = big.tile([P, ROWF], f32, tag="bufB")
            bufs = [buf_a, buf_b]
            nc.scalar.dma_start(out=yc[:], in_=ycols[:])
            nc.scalar.dma_start(out=hi[:], in_=hidx[:])
            nc.scalar.dma_start(out=hv[:], in_=hval[:])
            if gen_template:
                # Build the template on-device (~10 us, mostly DVE) instead
                # of streaming 2x4.2 MB template loads that would contend
                # with the store pipeline for HBM/DMA-engine bandwidth.
                # x channel: gpsimd iota (exact small ints in f32) + DVE
                # affine to xs[j]; y channels come from the per-tile y-copy
                # (enabled for t=0 below); buf_b's y is never read before
                # the tile-1 y-copy overwrites it.
                mult, add = mybir.AluOpType.mult, mybir.AluOpType.add
                nc.gpsimd.iota(buf_a[:, 2::4], pattern=[[1, G]], base=0,
                               channel_multiplier=0,
                               allow_small_or_imprecise_dtypes=True)
                nc.vector.memset(buf_a[:, 0::4], CORR_BASE)
                nc.vector.memset(buf_a[:, 1::4], CORR_BASE)
                nc.vector.tensor_scalar(
                    out=buf_a[:, 2::4], in0=buf_a[:, 2::4],
                    scalar1=float(DELTA_MAP), scalar2=float(MIN_W),
                    op0=mult, op1=add)
                nc.vector.memset(buf_b[:, 0::4], CORR_BASE)
                nc.vector.memset(buf_b[:, 1::4], CORR_BASE)
                nc.vector.tensor_copy(out=buf_b[:, 2::4], in_=buf_a[:, 2::4])
            else:
                # template loads split across 3 queues: store 0 only waits
                # on the two buf_a halves; buf_b (Pool SW queue) lands
                # under store 0.
                nc.sync.dma_start(out=buf_a[0:64, :], in_=btile[0:64, :])
                nc.scalar.dma_start(out=buf_a[64:128, :], in_=btile[64:128, :])
                nc.gpsimd.dma_start(out=buf_b[:], in_=btile[:])

            prev_last_sc = None
            for rep in range(repeat):
                stores = []
                for t in range(NT):
                    buf = bufs[t % 2]
                    rows = min(P, G - t * P)
                    # with gen_template the y channel is never pre-filled,
                    # so tile 0 needs the y-copy too on the first rep
                    if gen_template or t >= 1 or rep > 0:
                        nc.vector.tensor_copy(
                            out=buf[:, 3::4],
                            in_=yc[:, t:t + 1].to_broadcast([P, G]),
                        )
                    if store_queues == 1:
                        eng = nc.sync
                    elif store_queues == 2:
                        eng = (nc.sync, nc.scalar)[t % 2]
                    else:
                        eng = (nc.sync, nc.scalar, nc.gpsimd)[t % 3]
                    st = eng.dma_start(
                        out=out_rows[t * P:t * P + rows, :],
                        in_=buf[:rows, :],
                    )
                    if serialize_reps and prev_last_sc is not None and t == 0:
                        add_dep_helper(st.ins, prev_last_sc.ins)
                    stores.append(st)

                if not scatter:
                    prev_last_sc = stores[-1]
                    continue
                # HW DGE consumes ONE offset per partition and streams that
                # partition's whole in_ free dim contiguously from it — so
                # op o scatters 128 pairs (idx [128,1], payload [128,8] =
                # 32 B at byte address idx*16).  The declared out AP only
                # feeds the dependency tracker; writes land at the offsets.
                for o in range(NOPS):
                    t_dep = min(o, NT - 1) if banded else NT - 1
                    sc = nc.gpsimd.indirect_dma_start(
                        out=out[0:2],
                        out_offset=bass.IndirectOffsetOnAxis(
                            ap=hi[:, o:o + 1], axis=0),
                        in_=hv[:, 8 * o:8 * o + 8],
                        in_offset=None,
                    )
                    # op o touches rows only in tiles <= t_dep; stores on a
                    # FIFO queue complete in order, so waiting on the last
                    # `store_queues` stores <= t_dep covers every earlier
                    # store on every queue.
                    for d in range(max(store_queues, 2)):
                        if t_dep - d >= 0:
                            add_dep_helper(sc.ins, stores[t_dep - d].ins)
                    prev_last_sc = sc
    nc.finalize()
    return nc


def kernel(locs):
    global _NC_CACHE, LAST_RESULT
    from concourse.bass_utils import run_bass_kernel_spmd

    locs = np.asarray(locs, dtype=np.float32)
    assert locs.shape == (BATCH, N_LOCS, 2)

    _, ycols = _host_shared_inputs()
    in_maps = []
    banded = True
    for b in range(BATCH):
        hidx, hval = _host_scatter(locs[b])
        banded &= _host_scatter.last_banded_ok
        in_maps.append({"ycols": ycols, "hidx": hidx, "hval": hval})

    if _NC_CACHE is None or _NC_CACHE[1] != banded:
        _NC_CACHE = (_build_nc(banded=banded), banded)
    nc = _NC_CACHE[0]

    res = run_bass_kernel_spmd(nc, in_maps, core_ids=list(range(BATCH)),
                               trace=TRACE)
    LAST_RESULT = res
    outs = [res.results[b]["out"].reshape(G, G, 4) for b in range(BATCH)]
    return np.stack(outs, axis=0)


# revision 26
# speedup vs baseline: 2.4994x; 2.4994x over previous
"""Trainium2 kernel for nn_ContinuousLocationMap.

Contract: kernel(locs) with locs [8, 1024, 2] f32 -> [8, 2045, 2045, 4] f32.

Per batch item b the output grid is
    out[i, j] = hit(i,j) ? [1, 1, locs[b,w,0], locs[b,w,1]]
                         : [0.634, 0.634, xs[j], xs[i]]
where hit cells come from a 2x2 scatter window around each location index
and w is the last (max-ordinal) location that touched the cell.

Strategy (pure data parallel, one batch item per NeuronCore):
  host:   resolve the scatter winners (<= 4096 cells per item), cover the
          hit-cell set with ~2048 adjacent-cell PAIRS (32 B writes); every
          written half carries the cell's FINAL value, so overlapping or
          padded writes are order-independent.  Pairs are packed, in cell
          order, into NOPS=17 indirect-DMA ops of 128 slots such that op o
          only holds pairs whose rows lie in 128-row tiles <= min(o, 15).
  device: generate the base-map template on-device (~7 us on gpsimd+DVE,
          no 4.2 MB HBM template loads contending with stores), stream 16
          contiguous ~4.2 MB template stores at the write roofline
          (stride-4 y-channel rewrite per tile hidden on DVE), and patch
          hit cells with banded indirect DMAs: scatter op o waits only on
          the store of tile o, so patching hides under the remaining dense
          stores instead of tailing ~35 us after the last one.
"""

import numpy as np

# ---- hyperparameters (must mirror reference.py) ----
MIN_LOC = 0.0
MAX_LOC = 512.0
BINS = 2048
STRIDE = 1
WINDOW = 5

LOC_DELTA = (MAX_LOC - MIN_LOC) / BINS            # 0.25
WSIDE = WINDOW // 2                                # 2
BINS_WINDOW = BINS - 2 * WSIDE                     # 2044
MIN_W = MIN_LOC + LOC_DELTA * WSIDE                # 0.5
MAX_W = MIN_LOC + LOC_DELTA * BINS_WINDOW          # 511.0
G = int((BINS_WINDOW + 1) // STRIDE)               # 2045
DELTA_MAP = (MAX_W - MIN_W) / G
CORR_BASE = 0.634

BATCH = 8
N_LOCS = 1024

P = 128                       # SBUF partitions
ROWF = G * 4                  # 8180 floats per output row
NT = (G + P - 1) // P         # 16 row-tiles; last has 125 rows
NOPS = NT + 1                 # scatter ops: one per tile + one spill op
NPAIR = NOPS * P              # 2176 pair slots (>= 2048 = worst-case pairs)

TRACE = False                 # test.py sets this for profiling runs
LAST_RESULT = None            # BassKernelResults from the last run

# Bumped on every program-affecting edit.  Salts a dummy-output shape so
# the PJRT/NEFF compile-cache fingerprint can never serve a stale NEFF
# built from an older kernel version with identical tensor shapes.
KERNEL_VERSION = 6

_XS = (np.float32(MIN_W)
       + np.float32(DELTA_MAP) * np.arange(G, dtype=np.float32)).astype(np.float32)

_OFFS = np.array([[-1, -1], [-1, 0], [0, -1], [0, 0]], dtype=np.int32)


def _host_shared_inputs():
    """Template base tile (tile 0 content) and per-tile y columns."""
    btile = np.empty((P, ROWF), dtype=np.float32)
    btile[:, 0::4] = CORR_BASE
    btile[:, 1::4] = CORR_BASE
    btile[:, 2::4] = _XS[None, :]
    btile[:, 3::4] = _XS[:P, None]

    ycols = np.empty((P, NT), dtype=np.float32)
    for t in range(NT):
        rows = np.minimum(t * P + np.arange(P), G - 1)
        ycols[:, t] = _XS[rows]
    return btile, ycols


def _host_scatter(locs_b):
    """Pair-cover the last-write-wins scatter for one batch item.

    Returns (hidx [P, NOPS] int32, hval [P, NOPS*8] f32): op o scatters,
    per partition s, 8 floats hval[s, 8o:8o+8] to cells (c, c+1) where
    c = hidx[s, o].  Both halves always carry the cells' FINAL values
    (hit value or base-map value), so any overlap/padding is idempotent.
    Op o only contains pairs whose max touched row is in tile <= min(o,
    NT-1); the device schedules op o after the store of that tile.
    """
    locs_b = np.asarray(locs_b, dtype=np.float32)
    idx = (locs_b / np.float32(LOC_DELTA) / np.float32(STRIDE)).astype(np.int32)
    pos = idx[:, None, :] + _OFFS[None, :, :]                   # [L, 4, 2]
    valid = np.all((pos >= 0) & (pos <= G - 1), axis=-1)        # [L, 4]
    flat = np.where(valid, pos[..., 0] * G + pos[..., 1], 0)
    ordn = np.where(valid, np.arange(locs_b.shape[0], dtype=np.int64)[:, None], -1)

    winner = np.full(G * G, -1, dtype=np.int64)
    np.maximum.at(winner, flat.ravel(), ordn.ravel())
    cells = np.unique(flat[valid])
    cells = cells[winner[cells] >= 0]                           # ascending

    # greedy pair cover: lone trailing cell G*G-1 shifts left to stay
    # in-bounds (its left neighbour's final value is known anyway)
    c0s = []
    i, k = 0, len(cells)
    while i < k:
        c = int(cells[i])
        if i + 1 < k and cells[i + 1] == c + 1:
            c0s.append(c)
            i += 2
        else:
            c0s.append(c if c + 1 < G * G else c - 1)
            i += 1
    c0s = np.asarray(c0s + [0], dtype=np.int64)                 # +pad template
    npair = len(c0s) - 1
    assert npair <= NPAIR, npair

    def final_vals(cs):
        w = winner[cs]
        hit = w >= 0
        v = np.empty((len(cs), 4), np.float32)
        v[:, 0] = np.where(hit, np.float32(1.0), np.float32(CORR_BASE))
        v[:, 1] = v[:, 0]
        v[:, 2] = np.where(hit, locs_b[np.clip(w, 0, None), 0], _XS[cs % G])
        v[:, 3] = np.where(hit, locs_b[np.clip(w, 0, None), 1], _XS[cs // G])
        return v

    v8 = np.concatenate([final_vals(c0s), final_vals(c0s + 1)], axis=1)  # [n+1, 8]
    band = np.minimum(((c0s + 1) // G) // P, NT - 1)            # tile of max row

    # FIFO fill: op o takes the next <=128 pairs with band <= min(o, NT-1);
    # unused slots replay the pad pair (index npair: cell 0, final values).
    # If the banded schedule cannot place every pair (pathological row
    # clustering), fall back to unbanded fill (every op may hold any pair;
    # the device then runs every scatter after the last store).
    banded_ok = True
    sel = np.full((P, NOPS), npair, dtype=np.int64)
    ptr = 0
    for o in range(NOPS):
        bmax = min(o, NT - 1)
        take = 0
        while ptr < npair and take < P and band[ptr] <= bmax:
            sel[take, o] = ptr
            ptr += 1
            take += 1
    if ptr != npair:
        banded_ok = False
        sel = np.full((P, NOPS), npair, dtype=np.int64)
        order = np.arange(npair)
        sel[order % P, order // P] = order   # NOPS*P = 2176 >= max pairs

    hidx = c0s[sel].astype(np.int32)                            # [P, NOPS]
    # v8[sel] is [P, NOPS, 8]; hval[s, 8o:8o+8] = v8[sel[s, o]]
    hval = np.ascontiguousarray(v8[sel]).reshape(P, NOPS * 8).astype(np.float32)
    _host_scatter.last_banded_ok = banded_ok
    return hidx, hval


_NC_CACHE = None


def _build_nc(repeat=1, *, scatter=True, serialize_reps=False, store_queues=1,
              gen_template=True, banded=True):
    """Build the per-core Bass program (same program on all 8 cores).

    repeat>1 unrolls the whole store+scatter pipeline N times inside one
    NEFF (idempotent rewrites) — used by bench.py to isolate steady-state
    device time from the ~200ms per-call PJRT/axon dispatch overhead.
    Keyword-only flags are experiment knobs; defaults are the production
    config.
    """
    from concourse import bass, bacc, mybir
    import concourse.tile as tile
    from concourse.tile import add_dep_helper

    nc = bacc.Bacc(None, target_bir_lowering=False)
    f32 = mybir.dt.float32
    btile = None if gen_template else \
        nc.dram_tensor("btile", [P, ROWF], f32, kind="ExternalInput")
    ycols = nc.dram_tensor("ycols", [P, NT], f32, kind="ExternalInput")
    hidx = nc.dram_tensor("hidx", [P, NOPS], mybir.dt.int32, kind="ExternalInput")
    hval = nc.dram_tensor("hval", [P, NOPS * 8], f32, kind="ExternalInput")
    out = nc.dram_tensor("out", [G * G, 4], f32, kind="ExternalOutput")
    out_rows = out[:].rearrange("(g w) c -> g (w c)", w=G)      # [G, ROWF]
    # cache-buster: never-written dummy output whose shape encodes the
    # kernel version and build flags (see KERNEL_VERSION comment).
    salt = (KERNEL_VERSION * 1009 + repeat * 13 + int(scatter) * 7
            + int(serialize_reps) * 3 + store_queues
            + int(gen_template) * 29 + int(banded) * 61) % 8191 + 1
    nc.dram_tensor("vsalt", [1, salt], f32, kind="ExternalOutput")

    with tile.TileContext(nc) as tc:
        with tc.tile_pool(name="big", bufs=1) as big, \
             tc.tile_pool(name="small", bufs=1) as small:
            yc = small.tile([P, NT], f32, tag="yc")
            hi = small.tile([P, NOPS], mybir.dt.int32, tag="hi")
            hv = small.tile([P, NOPS * 8], f32, tag="hv")

            buf_a = big.tile([P, ROWF], f32, tag="bufA")
            buf_b = big.tile([P, ROWF], f32, tag="bufB")
            bufs = [buf_a, buf_b]
            nc.scalar.dma_start(out=yc[:], in_=ycols[:])
            nc.scalar.dma_start(out=hi[:], in_=hidx[:])
            nc.scalar.dma_start(out=hv[:], in_=hval[:])
            if gen_template:
                # Build the template on-device (~10 us, mostly DVE) instead
                # of streaming 2x4.2 MB template loads that would contend
                # with the store pipeline for HBM/DMA-engine bandwidth.
                # x channel: gpsimd iota (exact small ints in f32) + DVE
                # affine to xs[j]; y channels come from the per-tile y-copy
                # (enabled for t=0 below); buf_b's y is never read before
                # the tile-1 y-copy overwrites it.
                # Issue order IS engine order: everything store 0 needs
                # (all four buf_a channels) is emitted first, with the two
                # engines' chains balanced so buf_a is ready in ~5 us —
                # buf_b's ops must not sit between them on the DVE queue.
                mult, add = mybir.AluOpType.mult, mybir.AluOpType.add
                nc.gpsimd.iota(buf_a[:, 2::4], pattern=[[1, G]], base=0,
                               channel_multiplier=0,
                               allow_small_or_imprecise_dtypes=True)
                nc.gpsimd.memset(buf_a[:, 0::4], CORR_BASE)
                nc.vector.memset(buf_a[:, 1::4], CORR_BASE)
                nc.vector.tensor_copy(                   # tile-0 y channel
                    out=buf_a[:, 3::4],
                    in_=yc[:, 0:1].to_broadcast([P, G]))
                nc.vector.tensor_scalar(
                    out=buf_a[:, 2::4], in0=buf_a[:, 2::4],
                    scalar1=float(DELTA_MAP), scalar2=float(MIN_W),
                    op0=mult, op1=add)
                nc.vector.memset(buf_b[:, 0::4], CORR_BASE)
                nc.vector.memset(buf_b[:, 1::4], CORR_BASE)
                nc.vector.tensor_copy(out=buf_b[:, 2::4], in_=buf_a[:, 2::4])
            else:
                # template loads split across 3 queues: store 0 only waits
                # on the two buf_a halves; buf_b (Pool SW queue) lands
                # under store 0.
                nc.sync.dma_start(out=buf_a[0:64, :], in_=btile[0:64, :])
                nc.scalar.dma_start(out=buf_a[64:128, :], in_=btile[64:128, :])
                nc.gpsimd.dma_start(out=buf_b[:], in_=btile[:])

            prev_last_sc = None
            for rep in range(repeat):
                stores = []
                for t in range(NT):
                    buf = bufs[t % 2]
                    rows = min(P, G - t * P)
                    # tile 0's y channel is pre-filled (template gen block
                    # or btile load); later tiles/reps rewrite it
                    if t >= 1 or rep > 0:
                        nc.vector.tensor_copy(
                            out=buf[:, 3::4],
                            in_=yc[:, t:t + 1].to_broadcast([P, G]),
                        )
                    if store_queues == 1:
                        eng = nc.sync
                    elif store_queues == 2:
                        eng = (nc.sync, nc.scalar)[t % 2]
                    else:
                        eng = (nc.sync, nc.scalar, nc.gpsimd)[t % 3]
                    st = eng.dma_start(
                        out=out_rows[t * P:t * P + rows, :],
                        in_=buf[:rows, :],
                    )
                    if serialize_reps and prev_last_sc is not None and t == 0:
                        add_dep_helper(st.ins, prev_last_sc.ins)
                    stores.append(st)

                if not scatter:
                    prev_last_sc = stores[-1]
                    continue
                # HW DGE consumes ONE offset per partition and streams that
                # partition's whole in_ free dim contiguously from it — so
                # op o scatters 128 pairs (idx [128,1], payload [128,8] =
                # 32 B at byte address idx*16).  The declared out AP only
                # feeds the dependency tracker; writes land at the offsets.
                for o in range(NOPS):
                    t_dep = min(o, NT - 1) if banded else NT - 1
                    sc = nc.gpsimd.indirect_dma_start(
                        out=out[0:2],
                        out_offset=bass.IndirectOffsetOnAxis(
                            ap=hi[:, o:o + 1], axis=0),
                        in_=hv[:, 8 * o:8 * o + 8],
                        in_offset=None,
                    )
                    # op o touches rows only in tiles <= t_dep; stores on a
                    # FIFO queue complete in order, so waiting on the last
                    # `store_queues` stores <= t_dep covers every earlier
                    # store on every queue.
                    for d in range(max(store_queues, 2)):
                        if t_dep - d >= 0:
                            add_dep_helper(sc.ins, stores[t_dep - d].ins)
                    prev_last_sc = sc
    nc.finalize()
    return nc


def kernel(locs):
    global _NC_CACHE, LAST_RESULT
    from concourse.bass_utils import run_bass_kernel_spmd

    locs = np.asarray(locs, dtype=np.float32)
    assert locs.shape == (BATCH, N_LOCS, 2)

    _, ycols = _host_shared_inputs()
    in_maps = []
    banded = True
    for b in range(BATCH):
        hidx, hval = _host_scatter(locs[b])
        banded &= _host_scatter.last_banded_ok
        in_maps.append({"ycols": ycols, "hidx": hidx, "hval": hval})

    if _NC_CACHE is None or _NC_CACHE[1] != banded:
        _NC_CACHE = (_build_nc(banded=banded), banded)
    nc = _NC_CACHE[0]

    res = run_bass_kernel_spmd(nc, in_maps, core_ids=list(range(BATCH)),
                               trace=TRACE)
    LAST_RESULT = res
    outs = [res.results[b]["out"].reshape(G, G, 4) for b in range(BATCH)]
    return np.stack(outs, axis=0)


# revision 31
# speedup vs baseline: 3.4443x; 1.3780x over previous
"""Trainium2 kernel for nn_ContinuousLocationMap.

Contract: kernel(locs) with locs [8, 1024, 2] f32 -> [8, 2045, 2045, 4] f32.

Per batch item b the output grid is
    out[i, j] = hit(i,j) ? [1, 1, locs[b,w,0], locs[b,w,1]]
                         : [0.634, 0.634, xs[j], xs[i]]
where hit cells come from a 2x2 scatter window around each location index
and w is the last (max-ordinal) location that touched the cell.

Strategy (pure data parallel, one batch item per NeuronCore):
  host:   resolve the scatter winners (<= 4096 cells per item), cover the
          hit-cell set with ~2048 adjacent-cell PAIRS (32 B writes); every
          written half carries the cell's FINAL value, so overlapping or
          padded writes are order-independent.  Pairs are packed, in cell
          order, into NOPS=17 indirect-DMA ops of 128 slots such that op o
          only holds pairs whose rows lie in 128-row tiles <= min(o, 15).
  device: generate the base-map template on-device (~7 us on gpsimd+DVE,
          no 4.2 MB HBM template loads contending with stores), stream 16
          contiguous ~4.2 MB template stores at the write roofline
          (stride-4 y-channel rewrite per tile hidden on DVE), and patch
          hit cells with banded indirect DMAs: scatter op o waits only on
          the store of tile o, so patching hides under the remaining dense
          stores instead of tailing ~35 us after the last one.
"""

import numpy as np

# ---- hyperparameters (must mirror reference.py) ----
MIN_LOC = 0.0
MAX_LOC = 512.0
BINS = 2048
STRIDE = 1
WINDOW = 5

LOC_DELTA = (MAX_LOC - MIN_LOC) / BINS            # 0.25
WSIDE = WINDOW // 2                                # 2
BINS_WINDOW = BINS - 2 * WSIDE                     # 2044
MIN_W = MIN_LOC + LOC_DELTA * WSIDE                # 0.5
MAX_W = MIN_LOC + LOC_DELTA * BINS_WINDOW          # 511.0
G = int((BINS_WINDOW + 1) // STRIDE)               # 2045
DELTA_MAP = (MAX_W - MIN_W) / G
CORR_BASE = 0.634

BATCH = 8
N_LOCS = 1024

P = 128                       # SBUF partitions
ROWF = G * 4                  # 8180 floats per output row
NT = (G + P - 1) // P         # 16 row-tiles; last has 125 rows
NOPS = NT + 1                 # scatter ops: one per tile + one spill op
NPAIR = NOPS * P              # 2176 pair slots (>= 2048 = worst-case pairs)

TRACE = False                 # test.py sets this for profiling runs
LAST_RESULT = None            # BassKernelResults from the last run

# Bumped on every program-affecting edit.  Salts a dummy-output shape so
# the PJRT/NEFF compile-cache fingerprint can never serve a stale NEFF
# built from an older kernel version with identical tensor shapes.
KERNEL_VERSION = 7

# The last row-tile's store is split at this partition so the final
# scatter ops wait on a ~2 MB half-store instead of the whole tile:
# store units are [tile0..tile14, tile15[:64], tile15[64:]], and pair
# "bands" 0..16 index those units by the pair's max touched row.
SPLIT_P = 64
SPLIT_ROW = (NT - 1) * P + SPLIT_P - 1     # 1983: last row of unit 15

_XS = (np.float32(MIN_W)
       + np.float32(DELTA_MAP) * np.arange(G, dtype=np.float32)).astype(np.float32)

_OFFS = np.array([[-1, -1], [-1, 0], [0, -1], [0, 0]], dtype=np.int32)


def _host_shared_inputs():
    """Template base tile (tile 0 content) and per-tile y columns."""
    btile = np.empty((P, ROWF), dtype=np.float32)
    btile[:, 0::4] = CORR_BASE
    btile[:, 1::4] = CORR_BASE
    btile[:, 2::4] = _XS[None, :]
    btile[:, 3::4] = _XS[:P, None]

    ycols = np.empty((P, NT), dtype=np.float32)
    for t in range(NT):
        rows = np.minimum(t * P + np.arange(P), G - 1)
        ycols[:, t] = _XS[rows]
    return btile, ycols


def _host_scatter(locs_b):
    """Pair-cover the last-write-wins scatter for one batch item.

    Returns (hidx [P, NOPS] int32, hval [P, NOPS*8] f32): op o scatters,
    per partition s, 8 floats hval[s, 8o:8o+8] to cells (c, c+1) where
    c = hidx[s, o].  Both halves always carry the cells' FINAL values
    (hit value or base-map value), so any overlap/padding is idempotent.
    Op o only contains pairs whose max touched row is in tile <= min(o,
    NT-1); the device schedules op o after the store of that tile.
    """
    locs_b = np.asarray(locs_b, dtype=np.float32)
    idx = (locs_b / np.float32(LOC_DELTA) / np.float32(STRIDE)).astype(np.int32)
    pos = idx[:, None, :] + _OFFS[None, :, :]                   # [L, 4, 2]
    valid = np.all((pos >= 0) & (pos <= G - 1), axis=-1)        # [L, 4]
    flat = np.where(valid, pos[..., 0] * G + pos[..., 1], 0)
    ordn = np.where(valid, np.arange(locs_b.shape[0], dtype=np.int64)[:, None], -1)

    winner = np.full(G * G, -1, dtype=np.int64)
    np.maximum.at(winner, flat.ravel(), ordn.ravel())
    cells = np.unique(flat[valid])
    cells = cells[winner[cells] >= 0]                           # ascending

    # greedy pair cover: lone trailing cell G*G-1 shifts left to stay
    # in-bounds (its left neighbour's final value is known anyway)
    c0s = []
    i, k = 0, len(cells)
    while i < k:
        c = int(cells[i])
        if i + 1 < k and cells[i + 1] == c + 1:
            c0s.append(c)
            i += 2
        else:
            c0s.append(c if c + 1 < G * G else c - 1)
            i += 1
    c0s = np.asarray(c0s + [0], dtype=np.int64)                 # +pad template
    npair = len(c0s) - 1
    assert npair <= NPAIR, npair

    def final_vals(cs):
        w = winner[cs]
        hit = w >= 0
        v = np.empty((len(cs), 4), np.float32)
        v[:, 0] = np.where(hit, np.float32(1.0), np.float32(CORR_BASE))
        v[:, 1] = v[:, 0]
        v[:, 2] = np.where(hit, locs_b[np.clip(w, 0, None), 0], _XS[cs % G])
        v[:, 3] = np.where(hit, locs_b[np.clip(w, 0, None), 1], _XS[cs // G])
        return v

    v8 = np.concatenate([final_vals(c0s), final_vals(c0s + 1)], axis=1)  # [n+1, 8]
    maxrow = (c0s + 1) // G
    band = np.minimum(maxrow // P, NT - 1)          # store-unit of max row
    band = np.where((band == NT - 1) & (maxrow > SPLIT_ROW), NT, band)

    # FIFO fill: op o takes the next <=128 pairs with band <= o; unused
    # slots replay the pad pair (index npair: cell 0, final values).
    # If the banded schedule cannot place every pair (pathological row
    # clustering), fall back to unbanded fill (every op may hold any pair;
    # the device then runs every scatter after the last store).
    banded_ok = True
    sel = np.full((P, NOPS), npair, dtype=np.int64)
    ptr = 0
    for o in range(NOPS):
        bmax = o
        take = 0
        while ptr < npair and take < P and band[ptr] <= bmax:
            sel[take, o] = ptr
            ptr += 1
            take += 1
    if ptr != npair:
        banded_ok = False
        sel = np.full((P, NOPS), npair, dtype=np.int64)
        order = np.arange(npair)
        sel[order % P, order // P] = order   # NOPS*P = 2176 >= max pairs

    hidx = c0s[sel].astype(np.int32)                            # [P, NOPS]
    # v8[sel] is [P, NOPS, 8]; hval[s, 8o:8o+8] = v8[sel[s, o]]
    hval = np.ascontiguousarray(v8[sel]).reshape(P, NOPS * 8).astype(np.float32)
    _host_scatter.last_banded_ok = banded_ok
    return hidx, hval


_NC_CACHE = None


def _build_nc(repeat=1, *, scatter=True, serialize_reps=False, store_queues=1,
              gen_template=True, banded=True):
    """Build the per-core Bass program (same program on all 8 cores).

    repeat>1 unrolls the whole store+scatter pipeline N times inside one
    NEFF (idempotent rewrites) — used by bench.py to isolate steady-state
    device time from the ~200ms per-call PJRT/axon dispatch overhead.
    Keyword-only flags are experiment knobs; defaults are the production
    config.
    """
    from concourse import bass, bacc, mybir
    import concourse.tile as tile
    from concourse.tile import add_dep_helper

    nc = bacc.Bacc(None, target_bir_lowering=False)
    f32 = mybir.dt.float32
    btile = None if gen_template else \
        nc.dram_tensor("btile", [P, ROWF], f32, kind="ExternalInput")
    ycols = nc.dram_tensor("ycols", [P, NT], f32, kind="ExternalInput")
    hidx = nc.dram_tensor("hidx", [P, NOPS], mybir.dt.int32, kind="ExternalInput")
    hval = nc.dram_tensor("hval", [P, NOPS * 8], f32, kind="ExternalInput")
    out = nc.dram_tensor("out", [G * G, 4], f32, kind="ExternalOutput")
    out_rows = out[:].rearrange("(g w) c -> g (w c)", w=G)      # [G, ROWF]
    # cache-buster: never-written dummy output whose shape encodes the
    # kernel version and build flags (see KERNEL_VERSION comment).
    salt = (KERNEL_VERSION * 1009 + repeat * 13 + int(scatter) * 7
            + int(serialize_reps) * 3 + store_queues
            + int(gen_template) * 29 + int(banded) * 61) % 8191 + 1
    nc.dram_tensor("vsalt", [1, salt], f32, kind="ExternalOutput")

    with tile.TileContext(nc) as tc:
        with tc.tile_pool(name="big", bufs=1) as big, \
             tc.tile_pool(name="small", bufs=1) as small:
            yc = small.tile([P, NT], f32, tag="yc")
            hi = small.tile([P, NOPS], mybir.dt.int32, tag="hi")
            hv = small.tile([P, NOPS * 8], f32, tag="hv")

            buf_a = big.tile([P, ROWF], f32, tag="bufA")
            buf_b = big.tile([P, ROWF], f32, tag="bufB")
            bufs = [buf_a, buf_b]
            nc.scalar.dma_start(out=yc[:], in_=ycols[:])
            nc.scalar.dma_start(out=hi[:], in_=hidx[:])
            nc.scalar.dma_start(out=hv[:], in_=hval[:])
            if gen_template:
                # Build the template on-device (~10 us, mostly DVE) instead
                # of streaming 2x4.2 MB template loads that would contend
                # with the store pipeline for HBM/DMA-engine bandwidth.
                # x channel: gpsimd iota (exact small ints in f32) + DVE
                # affine to xs[j]; y channels come from the per-tile y-copy
                # (enabled for t=0 below); buf_b's y is never read before
                # the tile-1 y-copy overwrites it.
                # Issue order IS engine order: everything store 0 needs
                # (all four buf_a channels) is emitted first, with the two
                # engines' chains balanced so buf_a is ready in ~5 us —
                # buf_b's ops must not sit between them on the DVE queue.
                mult, add = mybir.AluOpType.mult, mybir.AluOpType.add
                nc.gpsimd.iota(buf_a[:, 2::4], pattern=[[1, G]], base=0,
                               channel_multiplier=0,
                               allow_small_or_imprecise_dtypes=True)
                nc.gpsimd.memset(buf_a[:, 0::4], CORR_BASE)
                nc.vector.memset(buf_a[:, 1::4], CORR_BASE)
                nc.vector.tensor_copy(                   # tile-0 y channel
                    out=buf_a[:, 3::4],
                    in_=yc[:, 0:1].to_broadcast([P, G]))
                nc.vector.tensor_scalar(
                    out=buf_a[:, 2::4], in0=buf_a[:, 2::4],
                    scalar1=float(DELTA_MAP), scalar2=float(MIN_W),
                    op0=mult, op1=add)
                nc.vector.memset(buf_b[:, 0::4], CORR_BASE)
                nc.vector.memset(buf_b[:, 1::4], CORR_BASE)
                nc.vector.tensor_copy(out=buf_b[:, 2::4], in_=buf_a[:, 2::4])
            else:
                # template loads split across 3 queues: store 0 only waits
                # on the two buf_a halves; buf_b (Pool SW queue) lands
                # under store 0.
                nc.sync.dma_start(out=buf_a[0:64, :], in_=btile[0:64, :])
                nc.scalar.dma_start(out=buf_a[64:128, :], in_=btile[64:128, :])
                nc.gpsimd.dma_start(out=buf_b[:], in_=btile[:])

            prev_last_sc = None
            for rep in range(repeat):
                stores = []
                for t in range(NT):
                    buf = bufs[t % 2]
                    rows = min(P, G - t * P)
                    # tile 0's y channel is pre-filled (template gen block
                    # or btile load); later tiles/reps rewrite it
                    if t >= 1 or rep > 0:
                        nc.vector.tensor_copy(
                            out=buf[:, 3::4],
                            in_=yc[:, t:t + 1].to_broadcast([P, G]),
                        )
                    if store_queues == 1:
                        eng = nc.sync
                    elif store_queues == 2:
                        eng = (nc.sync, nc.scalar)[t % 2]
                    else:
                        eng = (nc.sync, nc.scalar, nc.gpsimd)[t % 3]
                    # the last tile's store is split so the final scatter
                    # ops wait on a ~2 MB half instead of the whole tile
                    segs = ([(0, rows)] if t < NT - 1
                            else [(0, SPLIT_P), (SPLIT_P, rows)])
                    for r0, r1 in segs:
                        st = eng.dma_start(
                            out=out_rows[t * P + r0:t * P + r1, :],
                            in_=buf[r0:r1, :],
                        )
                        if serialize_reps and prev_last_sc is not None \
                                and t == 0:
                            add_dep_helper(st.ins, prev_last_sc.ins)
                        stores.append(st)

                if not scatter:
                    prev_last_sc = stores[-1]
                    continue
                # HW DGE consumes ONE offset per partition and streams that
                # partition's whole in_ free dim contiguously from it — so
                # op o scatters 128 pairs (idx [128,1], payload [128,8] =
                # 32 B at byte address idx*16).  The declared out AP only
                # feeds the dependency tracker; writes land at the offsets.
                for o in range(NOPS):       # len(stores) == NOPS == 17
                    t_dep = o if banded else NOPS - 1
                    sc = nc.gpsimd.indirect_dma_start(
                        out=out[0:2],
                        out_offset=bass.IndirectOffsetOnAxis(
                            ap=hi[:, o:o + 1], axis=0),
                        in_=hv[:, 8 * o:8 * o + 8],
                        in_offset=None,
                    )
                    # op o touches rows only in tiles <= t_dep; stores on a
                    # FIFO queue complete in order, so waiting on the last
                    # `store_queues` stores <= t_dep covers every earlier
                    # store on every queue.
                    for d in range(max(store_queues, 2)):
                        if t_dep - d >= 0:
                            add_dep_helper(sc.ins, stores[t_dep - d].ins)
                    prev_last_sc = sc
    nc.finalize()
    return nc


def kernel(locs):
    global _NC_CACHE, LAST_RESULT
    from concourse.bass_utils import run_bass_kernel_spmd

    locs = np.asarray(locs, dtype=np.float32)
    assert locs.shape == (BATCH, N_LOCS, 2)

    _, ycols = _host_shared_inputs()
    in_maps = []
    banded = True
    for b in range(BATCH):
        hidx, hval = _host_scatter(locs[b])
        banded &= _host_scatter.last_banded_ok
        in_maps.append({"ycols": ycols, "hidx": hidx, "hval": hval})

    if _NC_CACHE is None or _NC_CACHE[1] != banded:
        _NC_CACHE = (_build_nc(banded=banded), banded)
    nc = _NC_CACHE[0]

    res = run_bass_kernel_spmd(nc, in_maps, core_ids=list(range(BATCH)),
                               trace=TRACE)
    LAST_RESULT = res
    outs = [res.results[b]["out"].reshape(G, G, 4) for b in range(BATCH)]
    return np.stack(outs, axis=0)
